# revision 16
# baseline (speedup 1.0000x reference)
"""Multi-head attention (B=2, S=2048, D=1024, H=16) on 8 Trainium2 cores.

Sharding: tensor-parallel over heads for QKV+attention (2 heads/core),
then AllToAlls reshard the attention output so each core computes the
output projection for interleaved 64-token blocks (4 per batch).
Host assembles the full output from the per-core slices.

v2 restructure vs baseline:
  - token-major x^T streaming: K/V/Q chunks + the first attention unit's
    score groups are gated on 512-token DMA blocks, so attention starts
    ~6us in instead of waiting for the full 8MB load.
  - fused per-(b,t) attention units process both heads via 4-way
    quadrant-tiled score matmuls (2 heads x 2 key-halves on disjoint
    64x64 PE quadrants) -> scores at ~full array rate.
  - all batch-1 attention is emitted before any out-projection, so the
    batch-0 AllToAll latency hides behind ~85us of attention compute.
  - AllToAll split per unit-pair (A: t0/t1, B: t2/t3) with interleaved
    64-token output blocks; the final collective is covered by the
    out-projection of earlier chunks.
  - softmax normalize: reciprocal directly on the [1,512] denominator
    row; single DRAM round-trip for the partition broadcast.

PSUM budget (8 banks): tag s [128,1024]x2 = 4, tag av [65,512]x3 = 3,
tag o [128,512]x1 = 1. QKV/V/warmup/outproj borrow o.
"""
import os
import sys

sys.path.insert(0, "/opt/trn_rl_repo")

import numpy as np
import ml_dtypes

import concourse.bass as bass
import concourse.tile as tile
from concourse import bacc, mybir
from concourse import bass_utils

B = 2
S = 2048
D = 1024
H = 16
DH = 64
N_CORES = 8
HEADS_PER_CORE = H // N_CORES          # 2
S_SLICE = S // N_CORES                 # 256
N_CH = D // 128                        # 8 contraction chunks
N_QT = S // 512                        # 4 q tiles
N_KC = S // 128                        # 16 k chunks

F32 = mybir.dt.float32
BF16 = mybir.dt.bfloat16
FP8 = mybir.dt.float8e4
VSLOT = 72          # fp8 V slot: 64 dims + ones col + pad so the
                    # DoubleRow chunk-pair stride (2*72B) is 16B-aligned

_compiled = None
last_results = None


def _build():
    nc = bacc.Bacc(
        "TRN2",
        target_bir_lowering=False,
        debug=False,
        enable_asserts=True,
        num_devices=N_CORES,
    )

    xtb = nc.dram_tensor("xtb", [B, 128, N_CH, S], BF16, kind="ExternalInput").ap()
    wqt = nc.dram_tensor("wqt", [128, N_CH, 128], BF16, kind="ExternalInput").ap()
    wkt = nc.dram_tensor("wkt", [128, N_CH, 128], BF16, kind="ExternalInput").ap()
    wvt = nc.dram_tensor("wvt", [128, N_CH, 128], BF16, kind="ExternalInput").ap()
    wot = nc.dram_tensor("wot", [128, N_CH, D], BF16, kind="ExternalInput").ap()
    bb = nc.dram_tensor("bb", [128, D], F32, kind="ExternalInput").ap()
    oc = nc.dram_tensor("oc", [B, S_SLICE, D], F32, kind="ExternalOutput").ap()

    EXP = mybir.ActivationFunctionType.Exp
    SCALE = DH ** -0.5

    with tile.TileContext(nc) as tc:
        with (
            tc.tile_pool(name="w", bufs=1) as wp,
            tc.tile_pool(name="qkt", bufs=1) as qktp,
            tc.tile_pool(name="vsb", bufs=1) as vsbp,
            tc.tile_pool(name="xtb", bufs=2) as xtbp,
            tc.tile_pool(name="pt", bufs=2) as ptp,
            tc.tile_pool(name="norm", bufs=2) as normp,
            tc.tile_pool(name="x2", bufs=1) as x2p,
            tc.tile_pool(name="outsb", bufs=2) as outp,
            tc.tile_pool(name="dram", bufs=1, space="DRAM") as dram,
            tc.tile_pool(name="dramsc", bufs=4, space="DRAM") as dramsc,
            tc.tile_pool(name="sps", bufs=2, space="PSUM") as sps,
            tc.tile_pool(name="avps", bufs=3, space="PSUM") as avps,
            tc.tile_pool(name="ops", bufs=1, space="PSUM") as ops,
        ):
            # ---- weights ----
            wqt_sb = wp.tile([128, N_CH * 128], BF16)
            nc.sync.dma_start(wqt_sb[:], wqt[:].rearrange("p c e -> p (c e)"))
            wkt_sb = wp.tile([128, N_CH * 128], BF16)
            nc.sync.dma_start(wkt_sb[:], wkt[:].rearrange("p c e -> p (c e)"))
            wvt_sb = wp.tile([128, N_CH * 128], BF16)
            nc.sync.dma_start(wvt_sb[:], wvt[:].rearrange("p c e -> p (c e)"))

            # ---- starting gun: tiny AllGather aligns the 8 cores ----
            gun_in = dram.tile([1, 16], F32, name="gun_in")
            gun_out = dram.tile([N_CORES, 16], F32, name="gun_out")
            gun_sb = wp.tile([1, 16], F32)
            nc.gpsimd.memset(gun_sb[:], 0.0)
            nc.sync.dma_start(gun_in[:], gun_sb[:])
            nc.gpsimd.collective_compute(
                "AllGather", mybir.AluOpType.bypass,
                replica_groups=[list(range(N_CORES))],
                ins=[gun_in[:]], outs=[gun_out[:]],
            )

            # ---- PE warmup while DMAs stream ----
            warm = wp.tile([128, 512], BF16)
            nc.gpsimd.memset(warm[:], 0.0)
            for i in range(20):
                wps = ops.tile([128, 512], F32, tag="o", name="wps")
                nc.tensor.matmul(wps[:], lhsT=warm[:, 0:128], rhs=warm[:],
                                 start=True, stop=True)

            xtb_sbs = [None, None]
            Qt, Kt, Vs = [], [], []
            for b in range(B):
                Qt.append(qktp.tile([128, S], BF16, tag=f"qt{b}", name=f"qt{b}"))
                Kt.append(qktp.tile([128, S], BF16, tag=f"kt{b}", name=f"kt{b}"))
                Vs.append(vsbp.tile([128, N_KC * 2 * VSLOT], BF16, tag=f"v{b}",
                                    name=f"v{b}"))

            def emit_xtb_alloc(b):
                t_ = xtbp.tile([128, N_CH * S], BF16, tag="xtb", name="xtb_sb")
                xtb_sbs[b] = t_

            def emit_xtb_load_tb(b, tb):
                """Token-major: one 512-token block across all 8 ch chunks."""
                t_ = xtb_sbs[b]
                ts0 = tb * 512
                for ch in range(N_CH):
                    nc.sync.dma_start(
                        t_[:, ch * S + ts0:ch * S + ts0 + 512],
                        xtb[b, :, ch, ts0:ts0 + 512])

            def emit_qk_slice(b, t, which):
                """ch-inner, one [128,512] psum; K or Q for one t-tile."""
                w_sb, dst = (wqt_sb, Qt[b]) if which == "q" else (wkt_sb, Kt[b])
                ps_ = ops.tile([128, 512], F32, tag="o", name=f"{which}_ps1")
                for ch in range(N_CH):
                    nc.tensor.matmul(
                        ps_[:],
                        lhsT=w_sb[:, ch * 128:(ch + 1) * 128],
                        rhs=xtb_sbs[b][:, ch * S + t * 512:
                                       ch * S + (t + 1) * 512],
                        start=(ch == 0), stop=(ch == N_CH - 1),
                    )
                nc.vector.tensor_copy(dst[:, t * 512:(t + 1) * 512], ps_[:])

            def emit_v(b, sts):
                v_sb = Vs[b]
                for st in sts:
                    v_ps = ops.tile([128, 512], F32, tag="o", name="v_ps")
                    for ch in range(N_CH):
                        nc.tensor.matmul(
                            v_ps[:, 0:128],
                            lhsT=xtb_sbs[b][:, ch * S + st * 128:
                                            ch * S + (st + 1) * 128],
                            rhs=wvt_sb[:, ch * 128:(ch + 1) * 128],
                            start=(ch == 0), stop=(ch == N_CH - 1),
                        )
                    dst = v_sb[:].rearrange("p (c h o) -> p c h o",
                                            h=2, o=VSLOT)[:, st, :, 0:64]
                    nc.vector.tensor_copy(
                        dst, v_ps[:, 0:128].rearrange("p (h e) -> p h e", e=64)
                    )

            # a2a buffers: per (batch, half). Each [8 dst, 128 rows, 128 tok]
            # rows = 2 heads x 64 dims; tok = 64 from each unit of the pair.
            a2a_in = [[dram.tile([N_CORES, 128, 128], BF16, tag=f"a2ai{b}{h2}",
                                 name=f"a2ai{b}{h2}") for h2 in range(2)]
                      for b in range(B)]
            a2a_out = [[dram.tile([N_CORES, 128, 128], BF16, tag=f"a2ao{b}{h2}",
                                  name=f"a2ao{b}{h2}") for h2 in range(2)]
                       for b in range(B)]

            # ---- fused attention unit over both heads ----
            av_tiles = {}

            def att_groups(b, t, ccs, filler=None):
                """Score+exp+AV for key-chunk pairs ccs (each = 2x128 keys),
                both heads, query tile t."""
                qs = slice(t * 512, (t + 1) * 512)
                for cc in ccs:
                    if filler is not None:
                        filler()
                    if cc == 0:
                        av_tiles[(b, t, 0)] = avps.tile(
                            [65, 512], F32, tag="av", name="av0")
                        av_tiles[(b, t, 1)] = avps.tile(
                            [65, 512], F32, tag="av", name="av1")
                    s_tiles = []
                    for h in range(2):
                        s_tiles.append(sps.tile([128, 1024], F32, tag="s",
                                                name=f"s_h{h}"))
                    # 4-way quadrant tiling: (row=64h, col=64v)
                    for j in range(2):
                        c = 2 * cc + j
                        for h in range(2):
                            hp = slice(h * 64, (h + 1) * 64)
                            for v in range(2):
                                ks = slice(c * 128 + v * 64,
                                           c * 128 + v * 64 + 64)
                                nc.tensor.matmul(
                                    s_tiles[h][v * 64:(v + 1) * 64,
                                               j * 512:(j + 1) * 512],
                                    lhsT=Kt[b][hp, ks], rhs=Qt[b][hp, qs],
                                    start=True, stop=True,
                                    tile_position=(h * 64, v * 64),
                                )
                    for h in range(2):
                        p_sb = ptp.tile([128, 1024], BF16, tag="p",
                                        name=f"p_h{h}")
                        nc.scalar.activation(p_sb[:], s_tiles[h][:], EXP,
                                             scale=SCALE)
                        av = av_tiles[(b, t, h)]
                        for j in range(2):
                            c = 2 * cc + j
                            nc.tensor.matmul(
                                av[:],
                                lhsT=Vs[b][:].rearrange(
                                    "p (c2 h2 o) -> p c2 h2 o", h2=2, o=VSLOT
                                )[:, c, h, 0:65],
                                rhs=p_sb[:, j * 512:(j + 1) * 512],
                                start=(c == 0), stop=(c == N_KC - 1),
                                skip_group_check=True,
                            )

            def finish_unit(b, t):
                """Normalize both heads and write into the a2a buffer."""
                h2, tp = t // 2, t % 2
                for h in range(2):
                    av = av_tiles.pop((b, t, h))
                    # denom -> [64,8] reshape -> reciprocal -> broadcast
                    # ([1,512] reciprocal is 8 cyc/elem on ONE lane = 4us)
                    den_sb = normp.tile([1, 512], F32, tag="dsb", name="den_sb")
                    nc.vector.tensor_copy(den_sb[:], av[64:65, :])
                    den_d = dramsc.tile([512], F32, tag="dend", name="den_d")
                    nc.sync.dma_start(
                        den_d[:].rearrange("(a q) -> a q", a=1), den_sb[:])
                    den64 = normp.tile([64, 8], F32, tag="d64", name="den64")
                    nc.sync.dma_start(
                        den64[:], den_d[:].rearrange("(p q) -> p q", p=64))
                    rec64 = normp.tile([64, 8], F32, tag="r64", name="rec64")
                    nc.vector.reciprocal(rec64[:], den64[:])
                    rsc = dramsc.tile([512], F32, tag="rsc", name="rsc")
                    nc.sync.dma_start(
                        rsc[:].rearrange("(p q) -> p q", p=64), rec64[:])
                    bcast = normp.tile([64, 512], F32, tag="bc", name="bcast")
                    nc.sync.dma_start(
                        bcast[:],
                        rsc[:].rearrange("(a q) -> a q", a=1)
                        .broadcast_to([64, 512]),
                    )
                    o_sb = normp.tile([64, 512], BF16, tag="ob", name="o_sb")
                    nc.vector.tensor_mul(o_sb[:], av[0:64, :], bcast[:])
                    # scatter: token block u*64+j -> dst core u, col 64*tp+j
                    dst = a2a_in[b][h2][:, h * 64:(h + 1) * 64,
                                        tp * 64:(tp + 1) * 64]
                    nc.sync.dma_start(
                        dst.rearrange("u p j -> p u j"),
                        o_sb[:].rearrange("p (u j) -> p u j", u=N_CORES),
                    )

            def att_unit(b, t, filler=None):
                att_groups(b, t, range(N_KC // 2), filler)
                finish_unit(b, t)

            def emit_a2a(b, h2):
                nc.gpsimd.collective_compute(
                    "AllToAll", mybir.AluOpType.bypass,
                    replica_groups=[list(range(N_CORES))],
                    ins=[a2a_in[b][h2][:]], outs=[a2a_out[b][h2][:]],
                )

            x2_tiles = {}

            def emit_x2_loads(b, h2):
                x2_sb = x2p.tile([128, N_CORES * 128], BF16,
                                 tag=f"x2_{b}_{h2}", name=f"x2_{b}_{h2}")
                nc.sync.dma_start(
                    x2_sb[:].rearrange("p (u j) -> p u j", u=N_CORES),
                    a2a_out[b][h2][:].rearrange("u p j -> p u j"))
                x2_tiles[(b, h2)] = x2_sb

            def emit_outproj_piece(b, h2, et, wot_sb, bb_sb):
                o_ps = ops.tile([128, 512], F32, tag="o", name="o_ps")
                for ch in range(N_CH):
                    nc.tensor.matmul(
                        o_ps[:],
                        lhsT=x2_tiles[(b, h2)][:, ch * 128:(ch + 1) * 128],
                        rhs=wot_sb[:, ch * D + et * 512:ch * D + (et + 1) * 512],
                        start=(ch == 0), stop=(ch == N_CH - 1),
                    )
                out_sb = outp.tile([128, 512], F32, tag="osb", name="out_sb")
                nc.vector.tensor_add(
                    out_sb[:], o_ps[:], bb_sb[:, et * 512:(et + 1) * 512])
                nc.sync.dma_start(
                    oc[b, h2 * 128:(h2 + 1) * 128, et * 512:(et + 1) * 512],
                    out_sb[:],
                )

            # ================= pipeline =================
            ones0 = Vs[0][:].rearrange("p (s o) -> p s o", o=VSLOT)[:, :, 64:65]
            nc.gpsimd.memset(ones0, 1.0)
            ones1 = Vs[1][:].rearrange("p (s o) -> p s o", o=VSLOT)[:, :, 64:65]
            nc.gpsimd.memset(ones1, 1.0)

            # --- batch 0: streamed lead-in; attention unit t=0 follows the
            # token-major DMA blocks through the key dimension.
            emit_xtb_alloc(0)
            for tb in range(N_QT):
                emit_xtb_load_tb(0, tb)
            for tb in range(N_QT):
                emit_qk_slice(0, tb, "k")
                emit_qk_slice(0, tb, "q")
                emit_v(0, range(4 * tb, 4 * tb + 4))
                att_groups(0, 0, [2 * tb, 2 * tb + 1])
            finish_unit(0, 0)

            # --- batch-1 prep queue, pumped into batch-0 units t=1..3
            # (x loads just-in-time so 8MB of DMA issues don't clog the
            # sync queue ahead of batch-0's normalize/a2a writes)
            emit_xtb_alloc(1)
            prep = []
            for tb in range(N_QT):
                prep.append(lambda tb=tb: emit_xtb_load_tb(1, tb))
                prep.append(lambda tb=tb: emit_qk_slice(1, tb, "k"))
                prep.append(lambda tb=tb: emit_qk_slice(1, tb, "q"))
                for st in range(4 * tb, 4 * tb + 2):
                    prep.append(lambda st=st: emit_v(1, [st, st + 2]))

            def pump():
                if prep:
                    prep.pop(0)()

            att_unit(0, 1, filler=pump)
            emit_a2a(0, 0)
            emit_x2_loads(0, 0)
            att_unit(0, 2, filler=pump)
            att_unit(0, 3, filler=pump)
            while prep:
                prep.pop(0)()
            emit_a2a(0, 1)
            emit_x2_loads(0, 1)

            wot_sb = wp.tile([128, N_CH * D], BF16)
            nc.sync.dma_start(wot_sb[:], wot[:].rearrange("p c e -> p (c e)"))
            bb_sb = wp.tile([128, D], F32)
            nc.sync.dma_start(bb_sb[:], bb[:])

            # --- batch 1 attention, half B (units t2,t3) FIRST so the
            # half-B collective hides behind units t0,t1; the final
            # collective (half A) is covered by 6 outproj pieces.
            att_unit(1, 2)
            att_unit(1, 3)
            emit_a2a(1, 1)
            emit_x2_loads(1, 1)
            att_unit(1, 0)
            att_unit(1, 1)
            emit_a2a(1, 0)
            emit_x2_loads(1, 0)

            for et in range(2):
                emit_outproj_piece(0, 0, et, wot_sb, bb_sb)
            for et in range(2):
                emit_outproj_piece(0, 1, et, wot_sb, bb_sb)
            for et in range(2):
                emit_outproj_piece(1, 1, et, wot_sb, bb_sb)
            for et in range(2):
                emit_outproj_piece(1, 0, et, wot_sb, bb_sb)

    nc.compile()
    return nc


def _prep_chunked(a_t):
    """[Din, E] (already transposed) -> [128, Din//128, E] SBUF-chunk layout."""
    din, e = a_t.shape
    return np.ascontiguousarray(
        a_t.reshape(din // 128, 128, e).transpose(1, 0, 2)
    )


def kernel(x, w_qkv, w_out, b_out):
    global _compiled, last_results
    if _compiled is None:
        _compiled = _build()
    nc = _compiled

    x = np.asarray(x, dtype=np.float32)
    w_qkv = np.asarray(w_qkv, dtype=np.float32)
    w_out = np.asarray(w_out, dtype=np.float32)
    b_out = np.asarray(b_out, dtype=np.float32)

    # x^T in chunk layout: [B, 128, N_CH, S], bf16
    xt_full = x.transpose(0, 2, 1)  # [B, D, S]
    xtb_prep = np.ascontiguousarray(
        xt_full.reshape(B, N_CH, 128, S).transpose(0, 2, 1, 3)
    ).astype(ml_dtypes.bfloat16)

    wot_prep = _prep_chunked(np.ascontiguousarray(w_out.T)).astype(ml_dtypes.bfloat16)
    bb_np = np.ascontiguousarray(np.broadcast_to(b_out, (128, D)))

    in_maps = []
    for c in range(N_CORES):
        hA, hB = HEADS_PER_CORE * c, HEADS_PER_CORE * c + 1
        rows = np.r_[hA * DH:(hA + 1) * DH, hB * DH:(hB + 1) * DH]
        wq = w_qkv[rows, :]               # [128, D]
        wk = w_qkv[D + rows, :]
        wv = w_qkv[2 * D + rows, :]
        in_maps.append({
            "xtb": xtb_prep,
            "wqt": _prep_chunked(np.ascontiguousarray(wq.T)).astype(ml_dtypes.bfloat16),
            "wkt": _prep_chunked(np.ascontiguousarray(wk.T)).astype(ml_dtypes.bfloat16),
            "wvt": _prep_chunked(np.ascontiguousarray(wv.T)).astype(ml_dtypes.bfloat16),
            "wot": wot_prep,
            "bb": bb_np,
        })

    last_results = bass_utils.run_bass_kernel_spmd(
        nc, in_maps, core_ids=list(range(N_CORES))
    )
    # Interleaved unshard: core c's oc rows [64u : 64u+64] hold global
    # tokens [512u + 64c : 512u + 64c + 64] for u in 0..3, both batches.
    out = np.empty((B, S, D), dtype=np.float32)
    for c in range(N_CORES):
        occ = last_results.results[c]["oc"]
        for u in range(4):
            out[:, 512 * u + 64 * c:512 * u + 64 * c + 64, :] = \
                occ[:, 64 * u:64 * u + 64, :]
    return out


# revision 19
# speedup vs baseline: 1.0882x; 1.0882x over previous
"""Multi-head attention (B=2, S=2048, D=1024, H=16) on 8 Trainium2 cores.

Sharding: tensor-parallel over heads for QKV+attention (2 heads/core),
then AllToAlls reshard the attention output so each core computes the
output projection for interleaved 64-token blocks (4 per batch).
Host assembles the full output from the per-core slices.

v2 restructure vs baseline:
  - token-major x^T streaming: K/V/Q chunks + the first attention unit's
    score groups are gated on 512-token DMA blocks, so attention starts
    ~6us in instead of waiting for the full 8MB load.
  - fused per-(b,t) attention units process both heads via 4-way
    quadrant-tiled score matmuls (2 heads x 2 key-halves on disjoint
    64x64 PE quadrants) -> scores at ~full array rate.
  - all batch-1 attention is emitted before any out-projection, so the
    batch-0 AllToAll latency hides behind ~85us of attention compute.
  - AllToAll split per unit-pair (A: t0/t1, B: t2/t3) with interleaved
    64-token output blocks; the final collective is covered by the
    out-projection of earlier chunks.
  - softmax normalize: reciprocal directly on the [1,512] denominator
    row; single DRAM round-trip for the partition broadcast.

PSUM budget (8 banks): tag s [128,1024]x2 = 4, tag av [65,512]x3 = 3,
tag o [128,512]x1 = 1. QKV/V/warmup/outproj borrow o.
"""
import os
import sys

sys.path.insert(0, "/opt/trn_rl_repo")

import numpy as np
import ml_dtypes

import concourse.bass as bass
import concourse.tile as tile
from concourse import bacc, mybir
from concourse import bass_utils

B = 2
S = 2048
D = 1024
H = 16
DH = 64
N_CORES = 8
HEADS_PER_CORE = H // N_CORES          # 2
S_SLICE = S // N_CORES                 # 256
N_CH = D // 128                        # 8 contraction chunks
N_QT = S // 512                        # 4 q tiles
N_KC = S // 128                        # 16 k chunks

F32 = mybir.dt.float32
BF16 = mybir.dt.bfloat16
FP8 = mybir.dt.float8e4
VSLOT = 72          # fp8 V slot: 64 dims + ones col + pad so the
                    # DoubleRow chunk-pair stride (2*72B) is 16B-aligned

_compiled = None
last_results = None


def _build():
    nc = bacc.Bacc(
        "TRN2",
        target_bir_lowering=False,
        debug=False,
        enable_asserts=True,
        num_devices=N_CORES,
    )

    xtb = nc.dram_tensor("xtb", [B, 128, N_CH, S], BF16, kind="ExternalInput").ap()
    wqt = nc.dram_tensor("wqt", [128, N_CH, 128], BF16, kind="ExternalInput").ap()
    wkt = nc.dram_tensor("wkt", [128, N_CH, 128], BF16, kind="ExternalInput").ap()
    wvt = nc.dram_tensor("wvt", [128, N_CH, 128], BF16, kind="ExternalInput").ap()
    wot = nc.dram_tensor("wot", [128, N_CH, D], BF16, kind="ExternalInput").ap()
    bb = nc.dram_tensor("bb", [128, D], F32, kind="ExternalInput").ap()
    oc = nc.dram_tensor("oc", [B, S_SLICE, D], F32, kind="ExternalOutput").ap()

    EXP = mybir.ActivationFunctionType.Exp
    SCALE = DH ** -0.5

    with tile.TileContext(nc) as tc:
        with (
            tc.tile_pool(name="w", bufs=1) as wp,
            tc.tile_pool(name="qkt", bufs=1) as qktp,
            tc.tile_pool(name="vsb", bufs=1) as vsbp,
            tc.tile_pool(name="xtb", bufs=2) as xtbp,
            tc.tile_pool(name="pt", bufs=2) as ptp,
            tc.tile_pool(name="norm", bufs=2) as normp,
            tc.tile_pool(name="x2", bufs=1) as x2p,
            tc.tile_pool(name="outsb", bufs=2) as outp,
            tc.tile_pool(name="dram", bufs=1, space="DRAM") as dram,
            tc.tile_pool(name="dramsc", bufs=4, space="DRAM") as dramsc,
            tc.tile_pool(name="sps", bufs=2, space="PSUM") as sps,
            tc.tile_pool(name="avps", bufs=3, space="PSUM") as avps,
            tc.tile_pool(name="ops", bufs=1, space="PSUM") as ops,
        ):
            # ---- weights ----
            wqt_sb = wp.tile([128, N_CH * 128], BF16)
            nc.sync.dma_start(wqt_sb[:], wqt[:].rearrange("p c e -> p (c e)"))
            wkt_sb = wp.tile([128, N_CH * 128], BF16)
            nc.sync.dma_start(wkt_sb[:], wkt[:].rearrange("p c e -> p (c e)"))
            wvt_sb = wp.tile([128, N_CH * 128], BF16)
            nc.sync.dma_start(wvt_sb[:], wvt[:].rearrange("p c e -> p (c e)"))

            # ---- starting gun: tiny AllGather aligns the 8 cores ----
            gun_in = dram.tile([1, 16], F32, name="gun_in")
            gun_out = dram.tile([N_CORES, 16], F32, name="gun_out")
            gun_sb = wp.tile([1, 16], F32)
            nc.gpsimd.memset(gun_sb[:], 0.0)
            nc.sync.dma_start(gun_in[:], gun_sb[:])
            nc.gpsimd.collective_compute(
                "AllGather", mybir.AluOpType.bypass,
                replica_groups=[list(range(N_CORES))],
                ins=[gun_in[:]], outs=[gun_out[:]],
            )

            # ---- PE warmup while DMAs stream ----
            warm = wp.tile([128, 512], BF16)
            nc.gpsimd.memset(warm[:], 0.0)
            for i in range(20):
                wps = ops.tile([128, 512], F32, tag="o", name="wps")
                nc.tensor.matmul(wps[:], lhsT=warm[:, 0:128], rhs=warm[:],
                                 start=True, stop=True)

            xtb_sbs = [None, None]
            Qt, Kt, Vs = [], [], []
            for b in range(B):
                Qt.append(qktp.tile([128, S], BF16, tag=f"qt{b}", name=f"qt{b}"))
                Kt.append(qktp.tile([128, S], BF16, tag=f"kt{b}", name=f"kt{b}"))
                Vs.append(vsbp.tile([128, N_KC * 2 * VSLOT], BF16, tag=f"v{b}",
                                    name=f"v{b}"))

            def emit_xtb_alloc(b):
                t_ = xtbp.tile([128, N_CH * S], BF16, tag="xtb", name="xtb_sb")
                xtb_sbs[b] = t_

            def emit_xtb_load_tb(b, tb):
                """Token-major: one 512-token block across all 8 ch chunks,
                as a single strided DMA (sync-queue issue time matters)."""
                t_ = xtb_sbs[b]
                ts0 = tb * 512
                nc.sync.dma_start(
                    t_[:].rearrange("p (c s) -> p c s", s=S)[:, :, ts0:ts0 + 512],
                    xtb[b, :, :, ts0:ts0 + 512])

            def emit_qk_slice(b, t, which):
                """ch-inner, one [128,512] psum; K or Q for one t-tile."""
                w_sb, dst = (wqt_sb, Qt[b]) if which == "q" else (wkt_sb, Kt[b])
                ps_ = ops.tile([128, 512], F32, tag="o", name=f"{which}_ps1")
                for ch in range(N_CH):
                    nc.tensor.matmul(
                        ps_[:],
                        lhsT=w_sb[:, ch * 128:(ch + 1) * 128],
                        rhs=xtb_sbs[b][:, ch * S + t * 512:
                                       ch * S + (t + 1) * 512],
                        start=(ch == 0), stop=(ch == N_CH - 1),
                    )
                nc.vector.tensor_copy(dst[:, t * 512:(t + 1) * 512], ps_[:])

            def emit_v(b, sts):
                v_sb = Vs[b]
                for st in sts:
                    v_ps = ops.tile([128, 512], F32, tag="o", name="v_ps")
                    for ch in range(N_CH):
                        nc.tensor.matmul(
                            v_ps[:, 0:128],
                            lhsT=xtb_sbs[b][:, ch * S + st * 128:
                                            ch * S + (st + 1) * 128],
                            rhs=wvt_sb[:, ch * 128:(ch + 1) * 128],
                            start=(ch == 0), stop=(ch == N_CH - 1),
                        )
                    dst = v_sb[:].rearrange("p (c h o) -> p c h o",
                                            h=2, o=VSLOT)[:, st, :, 0:64]
                    nc.vector.tensor_copy(
                        dst, v_ps[:, 0:128].rearrange("p (h e) -> p h e", e=64)
                    )

            # a2a buffers: per (batch, half). Each [8 dst, 128 rows, 128 tok]
            # rows = 2 heads x 64 dims; tok = 64 from each unit of the pair.
            a2a_in = [[dram.tile([N_CORES, 128, 128], BF16, tag=f"a2ai{b}{h2}",
                                 name=f"a2ai{b}{h2}") for h2 in range(2)]
                      for b in range(B)]
            a2a_out = [[dram.tile([N_CORES, 128, 128], BF16, tag=f"a2ao{b}{h2}",
                                  name=f"a2ao{b}{h2}") for h2 in range(2)]
                       for b in range(B)]

            # ---- fused attention unit over both heads ----
            av_tiles = {}

            def att_groups(b, t, ccs, filler=None):
                """Score+exp+AV for key-chunk pairs ccs (each = 2x128 keys),
                both heads, query tile t."""
                qs = slice(t * 512, (t + 1) * 512)
                for cc in ccs:
                    if filler is not None:
                        filler()
                    if cc == 0:
                        av_tiles[(b, t, 0)] = avps.tile(
                            [65, 512], F32, tag="av", name="av0")
                        av_tiles[(b, t, 1)] = avps.tile(
                            [65, 512], F32, tag="av", name="av1")
                    s_tiles = []
                    for h in range(2):
                        s_tiles.append(sps.tile([128, 1024], F32, tag="s",
                                                name=f"s_h{h}"))
                    # 4-way quadrant tiling: (row=64h, col=64v)
                    for j in range(2):
                        c = 2 * cc + j
                        for h in range(2):
                            hp = slice(h * 64, (h + 1) * 64)
                            for v in range(2):
                                ks = slice(c * 128 + v * 64,
                                           c * 128 + v * 64 + 64)
                                nc.tensor.matmul(
                                    s_tiles[h][v * 64:(v + 1) * 64,
                                               j * 512:(j + 1) * 512],
                                    lhsT=Kt[b][hp, ks], rhs=Qt[b][hp, qs],
                                    start=True, stop=True,
                                    tile_position=(h * 64, v * 64),
                                )
                    last_p = None
                    for h in range(2):
                        p_sb = ptp.tile([128, 1024], BF16, tag="p",
                                        name=f"p_h{h}")
                        nc.scalar.activation(p_sb[:], s_tiles[h][:], EXP,
                                             scale=SCALE)
                        last_p = p_sb
                        av = av_tiles[(b, t, h)]
                        for j in range(2):
                            c = 2 * cc + j
                            nc.tensor.matmul(
                                av[:],
                                lhsT=Vs[b][:].rearrange(
                                    "p (c2 h2 o) -> p c2 h2 o", h2=2, o=VSLOT
                                )[:, c, h, 0:65],
                                rhs=p_sb[:, j * 512:(j + 1) * 512],
                                start=(c == 0), stop=(c == N_KC - 1),
                                skip_group_check=True,
                            )
                    # HAM keep-warm: one throwaway matmul per group, anchored
                    # on this group's exp output so the scheduler can't hoist
                    # it out of the idle window it is meant to fill.
                    wk = ops.tile([128, 512], F32, tag="o", name="wk")
                    nc.tensor.matmul(wk[:], lhsT=last_p[:, 0:128],
                                     rhs=last_p[:, 0:512],
                                     start=True, stop=True)

            def finish_unit(b, t):
                """Normalize both heads and write into the a2a buffer."""
                h2, tp = t // 2, t % 2
                avs = [av_tiles.pop((b, t, h)) for h in range(2)]
                # both heads' denominators in one [64,16] reshape ->
                # reciprocal -> one partition-broadcast (hop latency and
                # sync-queue issue time are the scarce resources here)
                den_sb = normp.tile([2, 512], F32, tag="dsb", name="den_sb")
                for h in range(2):
                    nc.vector.tensor_copy(den_sb[h:h + 1, :], avs[h][64:65, :])
                den_d = dramsc.tile([1024], F32, tag="dend", name="den_d")
                nc.sync.dma_start(
                    den_d[:].rearrange("(a q) -> a q", a=2), den_sb[:])
                den64 = normp.tile([64, 16], F32, tag="d64", name="den64")
                nc.sync.dma_start(
                    den64[:], den_d[:].rearrange("(p q) -> p q", p=64))
                rec64 = normp.tile([64, 16], F32, tag="r64", name="rec64")
                nc.vector.reciprocal(rec64[:], den64[:])
                rsc = dramsc.tile([1024], F32, tag="rsc", name="rsc")
                nc.sync.dma_start(
                    rsc[:].rearrange("(p q) -> p q", p=64), rec64[:])
                bcast = normp.tile([64, 1024], F32, tag="bc", name="bcast")
                nc.sync.dma_start(
                    bcast[:],
                    rsc[:].rearrange("(a q) -> a q", a=1)
                    .broadcast_to([64, 1024]),
                )
                for h in range(2):
                    o_sb = normp.tile([64, 512], BF16, tag="ob", name="o_sb")
                    nc.vector.tensor_mul(
                        o_sb[:], avs[h][0:64, :],
                        bcast[:, h * 512:(h + 1) * 512])
                    # scatter: token block u*64+j -> dst core u, col 64*tp+j
                    dst = a2a_in[b][h2][:, h * 64:(h + 1) * 64,
                                        tp * 64:(tp + 1) * 64]
                    nc.sync.dma_start(
                        dst.rearrange("u p j -> p u j"),
                        o_sb[:].rearrange("p (u j) -> p u j", u=N_CORES),
                    )

            def att_unit(b, t, filler=None):
                att_groups(b, t, range(N_KC // 2), filler)
                finish_unit(b, t)

            def emit_a2a(b, h2):
                nc.gpsimd.collective_compute(
                    "AllToAll", mybir.AluOpType.bypass,
                    replica_groups=[list(range(N_CORES))],
                    ins=[a2a_in[b][h2][:]], outs=[a2a_out[b][h2][:]],
                )

            x2_tiles = {}

            def emit_x2_loads(b, h2):
                x2_sb = x2p.tile([128, N_CORES * 128], BF16,
                                 tag=f"x2_{b}_{h2}", name=f"x2_{b}_{h2}")
                nc.sync.dma_start(
                    x2_sb[:].rearrange("p (u j) -> p u j", u=N_CORES),
                    a2a_out[b][h2][:].rearrange("u p j -> p u j"))
                x2_tiles[(b, h2)] = x2_sb

            def emit_outproj_piece(b, h2, et, wot_sb, bb_sb):
                o_ps = ops.tile([128, 512], F32, tag="o", name="o_ps")
                for ch in range(N_CH):
                    nc.tensor.matmul(
                        o_ps[:],
                        lhsT=x2_tiles[(b, h2)][:, ch * 128:(ch + 1) * 128],
                        rhs=wot_sb[:, ch * D + et * 512:ch * D + (et + 1) * 512],
                        start=(ch == 0), stop=(ch == N_CH - 1),
                    )
                out_sb = outp.tile([128, 512], F32, tag="osb", name="out_sb")
                nc.vector.tensor_add(
                    out_sb[:], o_ps[:], bb_sb[:, et * 512:(et + 1) * 512])
                nc.sync.dma_start(
                    oc[b, h2 * 128:(h2 + 1) * 128, et * 512:(et + 1) * 512],
                    out_sb[:],
                )

            # ================= pipeline =================
            ones0 = Vs[0][:].rearrange("p (s o) -> p s o", o=VSLOT)[:, :, 64:65]
            nc.gpsimd.memset(ones0, 1.0)
            ones1 = Vs[1][:].rearrange("p (s o) -> p s o", o=VSLOT)[:, :, 64:65]
            nc.gpsimd.memset(ones1, 1.0)

            # --- batch 0: streamed lead-in; attention unit t=0 follows the
            # token-major DMA blocks through the key dimension.
            emit_xtb_alloc(0)
            for tb in range(N_QT):
                emit_xtb_load_tb(0, tb)
            for tb in range(N_QT):
                emit_qk_slice(0, tb, "k")
                emit_qk_slice(0, tb, "q")
                emit_v(0, range(4 * tb, 4 * tb + 4))
                att_groups(0, 0, [2 * tb, 2 * tb + 1])
            finish_unit(0, 0)

            # --- batch-1 prep queue, pumped into batch-0 units t=1..3
            # (x loads just-in-time so 8MB of DMA issues don't clog the
            # sync queue ahead of batch-0's normalize/a2a writes)
            emit_xtb_alloc(1)
            prep = []
            for tb in range(N_QT):
                prep.append(lambda tb=tb: emit_xtb_load_tb(1, tb))
                prep.append(lambda tb=tb: emit_qk_slice(1, tb, "k"))
                prep.append(lambda tb=tb: emit_qk_slice(1, tb, "q"))
                for st in range(4 * tb, 4 * tb + 2):
                    prep.append(lambda st=st: emit_v(1, [st, st + 2]))

            def pump():
                if prep:
                    prep.pop(0)()

            att_unit(0, 1, filler=pump)
            emit_a2a(0, 0)
            emit_x2_loads(0, 0)
            att_unit(0, 2, filler=pump)
            att_unit(0, 3, filler=pump)
            while prep:
                prep.pop(0)()
            emit_a2a(0, 1)
            emit_x2_loads(0, 1)

            wot_sb = wp.tile([128, N_CH * D], BF16)
            nc.sync.dma_start(wot_sb[:], wot[:].rearrange("p c e -> p (c e)"))
            bb_sb = wp.tile([128, D], F32)
            nc.sync.dma_start(bb_sb[:], bb[:])

            # --- batch 1 attention, half B (units t2,t3) FIRST so the
            # half-B collective hides behind units t0,t1; the final
            # collective (half A) is covered by 6 outproj pieces.
            att_unit(1, 2)
            att_unit(1, 3)
            emit_a2a(1, 1)
            emit_x2_loads(1, 1)
            att_unit(1, 0)
            att_unit(1, 1)
            emit_a2a(1, 0)
            emit_x2_loads(1, 0)

            for et in range(2):
                emit_outproj_piece(0, 0, et, wot_sb, bb_sb)
            for et in range(2):
                emit_outproj_piece(0, 1, et, wot_sb, bb_sb)
            for et in range(2):
                emit_outproj_piece(1, 1, et, wot_sb, bb_sb)
            for et in range(2):
                emit_outproj_piece(1, 0, et, wot_sb, bb_sb)

    nc.compile()
    return nc


def _prep_chunked(a_t):
    """[Din, E] (already transposed) -> [128, Din//128, E] SBUF-chunk layout."""
    din, e = a_t.shape
    return np.ascontiguousarray(
        a_t.reshape(din // 128, 128, e).transpose(1, 0, 2)
    )


def kernel(x, w_qkv, w_out, b_out):
    global _compiled, last_results
    if _compiled is None:
        _compiled = _build()
    nc = _compiled

    x = np.asarray(x, dtype=np.float32)
    w_qkv = np.asarray(w_qkv, dtype=np.float32)
    w_out = np.asarray(w_out, dtype=np.float32)
    b_out = np.asarray(b_out, dtype=np.float32)

    # x^T in chunk layout: [B, 128, N_CH, S], bf16
    xt_full = x.transpose(0, 2, 1)  # [B, D, S]
    xtb_prep = np.ascontiguousarray(
        xt_full.reshape(B, N_CH, 128, S).transpose(0, 2, 1, 3)
    ).astype(ml_dtypes.bfloat16)

    wot_prep = _prep_chunked(np.ascontiguousarray(w_out.T)).astype(ml_dtypes.bfloat16)
    bb_np = np.ascontiguousarray(np.broadcast_to(b_out, (128, D)))

    in_maps = []
    for c in range(N_CORES):
        hA, hB = HEADS_PER_CORE * c, HEADS_PER_CORE * c + 1
        rows = np.r_[hA * DH:(hA + 1) * DH, hB * DH:(hB + 1) * DH]
        wq = w_qkv[rows, :]               # [128, D]
        wk = w_qkv[D + rows, :]
        wv = w_qkv[2 * D + rows, :]
        in_maps.append({
            "xtb": xtb_prep,
            "wqt": _prep_chunked(np.ascontiguousarray(wq.T)).astype(ml_dtypes.bfloat16),
            "wkt": _prep_chunked(np.ascontiguousarray(wk.T)).astype(ml_dtypes.bfloat16),
            "wvt": _prep_chunked(np.ascontiguousarray(wv.T)).astype(ml_dtypes.bfloat16),
            "wot": wot_prep,
            "bb": bb_np,
        })

    last_results = bass_utils.run_bass_kernel_spmd(
        nc, in_maps, core_ids=list(range(N_CORES))
    )
    # Interleaved unshard: core c's oc rows [64u : 64u+64] hold global
    # tokens [512u + 64c : 512u + 64c + 64] for u in 0..3, both batches.
    out = np.empty((B, S, D), dtype=np.float32)
    for c in range(N_CORES):
        occ = last_results.results[c]["oc"]
        for u in range(4):
            out[:, 512 * u + 64 * c:512 * u + 64 * c + 64, :] = \
                occ[:, 64 * u:64 * u + 64, :]
    return out
